# revision 17
# baseline (speedup 1.0000x reference)
"""Self-contained 2-layer GAT kernel for Trainium2, 8-core SPMD.

Strategy: edges sharded by destination node across the 8 cores (each core
owns a contiguous dst slice; edges sorted by dst tile on host). The node
phase (x@W) is replicated on every core into a bf16 DRAM table (256B rows,
c-major head interleave) so the edge phase gathers 256B per edge. The tiny
attention projections (x @ W @ a_src/dst, ~50 MFLOP) are computed on host
and shipped as a pre-added per-edge-slot alpha table, which removes the
per-edge aS/aD gathers entirely. Bias-add + ReLU + head un-interleave run
on host between the two launches (b is folded there).

Per layer, per core:
  node phase:  h = xT_chunk.T @ Wperm (PE, bf16) -> htab rows (128 bf16)
  edge phase (per 5-tile group, per 128-dst tile):
      dma_gather h rows by src (int16 idx; lo/hi half tables)
      ex = exp(lrelu(alpha_pre))                        (DVE+ACT, bf16)
      Ind[e,d] = (iota == dstloc[e]) one-hot            (DVE/Pool, bf16)
      PSUM accum: [out|denom] += Ind.T @ [ex*h | ex]    (PE, bf16)
      epilogue: out/denom -> output rows (f32, pre-bias pre-relu).
"""

import sys
import numpy as np
import ml_dtypes

sys.path.insert(0, "/opt/trn_rl_repo")

import concourse.bacc as bacc
import concourse.mybir as mybir
from concourse.bass_utils import run_bass_kernel_spmd
from concourse.tile import TileContext

f32 = mybir.dt.float32
bf16 = mybir.dt.bfloat16
i16 = mybir.dt.int16
i32 = mybir.dt.int32
npbf16 = ml_dtypes.bfloat16

P = 128
H = 4
C = 32
F = 128          # feature width (= H*C)
FA = F + H       # msgex width: h | ex
GS = 5           # dst tiles per gather group
IND_SPLIT = 8    # of every 13 ind builds, this many go to DVE (rest Pool)

N_CORES = 8
N_NODES = 50000
N_EDGES = 800000

# device column order is c-major: dev col c*H+h <-> ref col h*C+c
PERM = np.arange(F).reshape(H, C).T.flatten()      # ref col for each dev col
IPERM = np.arange(F).reshape(C, H).T.flatten()     # dev col for each ref col

import os
_SKIP = set(os.environ.get("GAT_SKIP", "").split(","))  # perf-bisect flags


def _make_plan(src, dst, N, n_cores):
    npad = ((N + P * n_cores - 1) // (P * n_cores)) * (P * n_cores)
    npc = npad // n_cores
    NT = npc // P
    NCH = npad // P
    NLO = npad // 2                # nodes in the lo half table
    assert NLO <= 32767 and NLO % P == 0

    tile_of = dst // P             # global dst chunk id
    src_hi = (src >= NLO).astype(np.int64)

    cnt = np.zeros((NCH, 2), np.int64)
    np.add.at(cnt, (tile_of, src_hi), 1)
    cnt_ct = cnt.reshape(n_cores, NT, 2)
    Klo = np.maximum(1, np.ceil(cnt_ct[:, :, 0].max(axis=0) / P).astype(np.int64))
    Khi = np.ceil(cnt_ct[:, :, 1].max(axis=0) / P).astype(np.int64)
    LOCH = int(Klo.sum())
    HICH = int(Khi.sum())
    TOTCH = LOCH + HICH
    CO_lo = np.concatenate([[0], np.cumsum(Klo)])[:-1]
    CO_hi = LOCH + np.concatenate([[0], np.cumsum(Khi)])[:-1]

    okey = tile_of * 2 + src_hi
    order = np.argsort(okey, kind="stable")
    s_src = src[order]
    s_dst = dst[order]
    s_key = okey[order]
    starts = np.searchsorted(s_key, np.arange(NCH * 2))
    ends = np.searchsorted(s_key, np.arange(NCH * 2) + 1)

    gsrci = np.zeros((n_cores, 16, 8 * TOTCH), np.int16)
    gloc = np.full((n_cores, P, TOTCH), float(P), np.float32)
    gsrcn = np.zeros((n_cores, P, TOTCH), np.int32)   # global src node (pads 0)
    gdstn = np.zeros((n_cores, P, TOTCH), np.int32)   # global dst node (pads 0)
    gpad = np.ones((n_cores, P, TOTCH), bool)

    for c in range(n_cores):
        for t in range(NT):
            g = c * NT + t
            for half, co, nk in ((0, CO_lo[t], Klo[t]), (1, CO_hi[t], Khi[t])):
                if nk == 0:
                    continue
                e0, e1 = starts[2 * g + half], ends[2 * g + half]
                n = e1 - e0
                npadn = int(nk) * P
                sv = np.zeros(npadn, np.int64)
                lv = np.full(npadn, P, np.int64)
                sn = np.zeros(npadn, np.int64)
                dn = np.zeros(npadn, np.int64)
                if n:
                    ev = s_src[e0:e1]
                    sv[:n] = np.where(ev >= NLO, ev - NLO, ev)
                    lv[:n] = s_dst[e0:e1] % P
                    sn[:n] = ev
                    dn[:n] = s_dst[e0:e1]
                j = np.arange(npadn)
                cc = 8 * int(co) + j // 16
                rr = j % 16
                gsrci[c, rr, cc] = sv
                kk = int(co) + j // P
                pp = j % P
                gloc[c, pp, kk] = lv
                gsrcn[c, pp, kk] = sn
                gdstn[c, pp, kk] = dn
                gpad[c, pp[:n], kk[:n]] = False

    gsrci = np.tile(gsrci, (1, 8, 1))

    groups = [(t0, min(t0 + GS, NT)) for t0 in range(0, NT, GS)]

    return dict(
        n_cores=n_cores, N=N, npad=npad, npc=npc, NT=NT, NCH=NCH, NLO=NLO,
        Klo=[int(k) for k in Klo], Khi=[int(k) for k in Khi],
        LOCH=LOCH, HICH=HICH, TOTCH=TOTCH,
        CO_lo=[int(o) for o in CO_lo], CO_hi=[int(o) for o in CO_hi],
        groups=groups,
        gsrci=gsrci, gloc=gloc, gsrcn=gsrcn, gdstn=gdstn, gpad=gpad,
    )


def _layer_inputs(plan, x, W, a_src, a_dst):
    """x: [npad, F] f32 (rows >= N zero). Returns per-core input maps."""
    npad, NCH, TOTCH = plan["npad"], plan["NCH"], plan["TOTCH"]
    W = np.asarray(W, np.float32)
    Ablk_s = np.zeros((F, H), np.float32)
    Ablk_d = np.zeros((F, H), np.float32)
    for h in range(H):
        Ablk_s[h * C:(h + 1) * C, h] = a_src[h]
        Ablk_d[h * C:(h + 1) * C, h] = a_dst[h]

    xt = np.ascontiguousarray(
        x.reshape(NCH, P, F).transpose(0, 2, 1)).astype(npbf16)
    wperm = W[:, PERM].astype(npbf16)

    aS = x @ (W @ Ablk_s)          # [npad, H] f32
    aD = x @ (W @ Ablk_d)
    alpha = aS[plan["gsrcn"]] + aD[plan["gdstn"]]   # [cores, P, TOTCH, H]
    alpha[plan["gpad"]] = 0.0
    alpha = alpha.astype(npbf16)

    return [
        dict(xt=xt, wcat=wperm, gsrci=plan["gsrci"][c],
             galpha=alpha[c], gloc=plan["gloc"][c])
        for c in range(plan["n_cores"])
    ]


def _build_layer_kernel(plan):
    NT, NCH, TOTCH, NLO = plan["NT"], plan["NCH"], plan["TOTCH"], plan["NLO"]
    Klo, Khi = plan["Klo"], plan["Khi"]
    CO_lo, CO_hi = plan["CO_lo"], plan["CO_hi"]
    npad = plan["npad"]

    nc = bacc.Bacc()
    xt = nc.dram_tensor("xt", [NCH, F, P], bf16, kind="ExternalInput")
    wcat = nc.dram_tensor("wcat", [F, F], bf16, kind="ExternalInput")
    gsrci = nc.dram_tensor("gsrci", [P, 8 * TOTCH], i16, kind="ExternalInput")
    galpha = nc.dram_tensor("galpha", [P, TOTCH, H], bf16, kind="ExternalInput")
    gloc = nc.dram_tensor("gloc", [P, TOTCH], f32, kind="ExternalInput")
    out = nc.dram_tensor("out", [NT * P, F], f32, kind="ExternalOutput")

    htab = nc.dram_tensor("htab", [npad, F], bf16)

    # Phase 1: node phase (own TileContext; its exit barrier guarantees htab
    # is fully in DRAM before any edge-phase gather issues).
    with TileContext(nc) as tc:
        with (
            tc.tile_pool(name="const", bufs=1) as cpool,
            tc.tile_pool(name="nodein", bufs=4) as npool,
            tc.tile_pool(name="nodeout", bufs=4) as hpool,
            tc.tile_pool(name="npsum", bufs=4, space="PSUM") as npsum,
        ):
            wcat_sb = cpool.tile([F, F], bf16)
            nc.sync.dma_start(wcat_sb[:, :], wcat[:, :])

            NB = 24
            node_batches = [] if "node" in _SKIP else [
                (b, min(NB, NCH - b)) for b in range(0, NCH, NB)
            ]
            cpy = 0
            for bi, (b, nb) in enumerate(node_batches):
                xcb = npool.tile([F, NB, P], bf16, tag="xc")
                nc.sync.dma_start(
                    xcb[:, 0:nb, :],
                    xt[b:b + nb, :, :].rearrange("n f p -> f n p"))
                hcb = hpool.tile([P, NB, F], bf16, tag="hc")
                for k4 in range(0, nb, 4):
                    kk = min(4, nb - k4)
                    ps = npsum.tile([P, 4 * F], f32, tag="nps")
                    for k in range(kk):
                        nc.tensor.matmul(
                            ps[:, k * F:(k + 1) * F], lhsT=xcb[:, k4 + k, :],
                            rhs=wcat_sb[:, :], start=True, stop=True)
                    dst_ap = hcb[:, k4:k4 + kk, :]
                    src_ap = ps[:, 0:kk * F].rearrange("p (k f) -> p k f", f=F)
                    nc.vector.tensor_copy(dst_ap, src_ap)
                    cpy += 1
                # htab store via the Act HWDGE queue (separate from xcb's SP)
                nc.scalar.dma_start(
                    htab[b * P:(b + nb) * P, :].rearrange(
                        "(n p) w -> p n w", p=P),
                    hcb[:, 0:nb, :])

    # Phase 2: edge phase.
    with TileContext(nc) as tc:
        with (
            tc.tile_pool(name="econst", bufs=1) as cpool,
            tc.tile_pool(name="egather", bufs=2) as gpool,
            tc.tile_pool(name="eex", bufs=2) as epool,
            tc.tile_pool(name="emsg", bufs=2) as mpool,
            tc.tile_pool(name="eind", bufs=2) as ipool,
            tc.tile_pool(name="epsum", bufs=8, space="PSUM") as epsum,
            tc.tile_pool(name="eout", bufs=3) as opool,
        ):
            iota_i = cpool.tile([P, P], i32)
            nc.gpsimd.iota(iota_i[:, :], pattern=[[1, P]], base=0,
                           channel_multiplier=0)
            iota_f = cpool.tile([P, P], bf16)
            nc.vector.tensor_copy(iota_f[:, :], iota_i[:, :])

            srcA = cpool.tile([P, 8 * TOTCH], i16)
            nc.sync.dma_start(srcA[:, :], gsrci[:, :])
            locA = cpool.tile([P, TOTCH], f32)
            nc.sync.dma_start(locA[:, :], gloc[:, :])
            galA = cpool.tile([P, TOTCH, H], bf16)
            nc.sync.dma_start(galA[:, :, :], galpha[:, :, :])

            indcnt = 0
            for g0, g1 in plan["groups"]:
                if "edge" in _SKIP:
                    break
                clo0 = CO_lo[g0]
                clo1 = CO_lo[g1 - 1] + Klo[g1 - 1]
                chi0 = CO_hi[g0]
                chi1 = CO_hi[g1 - 1] + Khi[g1 - 1]
                nlo, nhi = clo1 - clo0, chi1 - chi0
                ng = nlo + nhi

                halves = []
                hsa_lo = gpool.tile([P, nlo, F], bf16, tag="hlo")
                nc.gpsimd.dma_gather(
                    out_ap=hsa_lo[:, :, :], in_ap=htab[0:NLO, :],
                    idxs_ap=srcA[:, 8 * clo0:8 * clo1],
                    num_idxs=nlo * P, num_idxs_reg=nlo * P, elem_size=F,
                    single_packet=False)
                halves.append((hsa_lo, clo0, nlo))
                if nhi > 0:
                    hsa_hi = gpool.tile([P, nhi, F], bf16, tag="hhi")
                    nc.gpsimd.dma_gather(
                        out_ap=hsa_hi[:, :, :], in_ap=htab[NLO:npad, :],
                        idxs_ap=srcA[:, 8 * chi0:8 * chi1],
                        num_idxs=nhi * P, num_idxs_reg=nhi * P, elem_size=F,
                        single_packet=False)
                    halves.append((hsa_hi, chi0, nhi))

                # one-hot builds for the whole group: no data deps on the
                # gathers, so they fill the gather latency on DVE/Pool
                indg = ipool.tile([P, ng, P], bf16, tag="ind")
                for j in range(ng):
                    co = (clo0 + j) if j < nlo else (chi0 + j - nlo)
                    eng = (nc.vector if indcnt % 13 < IND_SPLIT
                           else nc.gpsimd)
                    eng.tensor_scalar(
                        out=indg[:, j, :], in0=iota_f[:, :],
                        scalar1=locA[:, co:co + 1],
                        scalar2=None, op0=mybir.AluOpType.is_equal)
                    indcnt += 1

                msgs = []
                for hsa, c0, nch in halves:
                    lrl = epool.tile([P, nch, H], bf16, tag="lrl")
                    nc.vector.scalar_tensor_tensor(
                        out=lrl[:, :, :], in0=galA[:, c0:c0 + nch, :],
                        scalar=0.2, in1=galA[:, c0:c0 + nch, :],
                        op0=mybir.AluOpType.mult, op1=mybir.AluOpType.max)
                    ex = epool.tile([P, nch, H], bf16, tag="ex")
                    nc.scalar.activation(ex[:, :, :], lrl[:, :, :],
                                         mybir.ActivationFunctionType.Exp)
                    m = mpool.tile([P, nch, FA], bf16, tag="msg")
                    nc.vector.tensor_tensor(
                        out=m[:, :, 0:F].rearrange("p k (c h) -> p k c h", h=H),
                        in0=hsa[:, :, :].rearrange("p k (c h) -> p k c h", h=H),
                        in1=ex[:, :, :].rearrange(
                            "p k (o h) -> p k o h", o=1).to_broadcast(
                            [P, nch, C, H]),
                        op=mybir.AluOpType.mult)
                    nc.scalar.copy(m[:, :, F:FA], ex[:, :, :])
                    msgs.append((m, c0, nch))

                for t in range(g0, g1):
                    pso = epsum.tile([P, FA], f32, tag="pso")
                    nk = Klo[t] + Khi[t]
                    ki = 0
                    for hv, (m, c0, nch) in enumerate(msgs):
                        co = CO_lo[t] if hv == 0 else CO_hi[t]
                        cnt = Klo[t] if hv == 0 else Khi[t]
                        off = co - c0
                        goff = (co - clo0) if hv == 0 else (nlo + co - chi0)
                        for k in range(cnt):
                            nc.tensor.matmul(
                                pso[:, :], lhsT=indg[:, goff + k, :],
                                rhs=m[:, off + k, :],
                                start=(ki == 0), stop=(ki == nk - 1))
                            ki += 1

                    den = opool.tile([P, H], f32, tag="den")
                    nc.scalar.activation(den[:, :], pso[:, F:FA],
                                         mybir.ActivationFunctionType.Copy,
                                         bias=1e-16)
                    rec = opool.tile([P, H], f32, tag="rec")
                    nc.vector.reciprocal(rec[:, :], den[:, :])
                    on = opool.tile([P, F], f32, tag="on")
                    nc.vector.tensor_tensor(
                        out=on[:, :].rearrange("p (c h) -> p c h", h=H),
                        in0=pso[:, 0:F].rearrange("p (c h) -> p c h", h=H),
                        in1=rec[:, :].rearrange(
                            "p (o h) -> p o h", o=1).to_broadcast([P, C, H]),
                        op=mybir.AluOpType.mult)
                    nc.sync.dma_start(out[t * P:(t + 1) * P, :], on[:, :])
            if "edge" in _SKIP:
                zo = cpool.tile([P, F], f32)
                nc.vector.memset(zo[:, :], 0.0)
                for t in range(NT):
                    nc.sync.dma_start(out[t * P:(t + 1) * P, :], zo[:, :])

    nc.finalize()
    return nc


_KERNEL_CACHE = {}


def _get_kernel(plan):
    key = (tuple(plan["Klo"]), tuple(plan["Khi"]), plan["npad"])
    if key not in _KERNEL_CACHE:
        _KERNEL_CACHE[key] = _build_layer_kernel(plan)
    return _KERNEL_CACHE[key]


def _run_layer(nc, maps, trace=False):
    last = None
    for attempt in range(3):
        try:
            res = run_bass_kernel_spmd(nc, maps, list(range(len(maps))),
                                       trace=trace)
            outs = [r["out"] for r in res.results]
            return np.concatenate(outs, axis=0), res
        except Exception as e:  # transient NRT_EXEC_UNIT_UNRECOVERABLE etc.
            last = e
            import time as _time
            _time.sleep(2.0 * (attempt + 1))
    raise last


def kernel(x, edge_index, W1, a_src1, a_dst1, b1, W2, a_src2, a_dst2, b2,
           _trace=False, _collect=None):
    x = np.asarray(x, dtype=np.float32)
    edge_index = np.asarray(edge_index)
    assert x.shape == (N_NODES, F), x.shape
    assert edge_index.shape == (2, N_EDGES), edge_index.shape

    loops = np.arange(N_NODES, dtype=np.int64)
    src = np.concatenate([edge_index[0].astype(np.int64), loops])
    dst = np.concatenate([edge_index[1].astype(np.int64), loops])

    plan = _make_plan(src, dst, N_NODES, N_CORES)
    nc = _get_kernel(plan)
    npad = plan["npad"]

    xp = np.zeros((npad, F), np.float32)
    xp[:N_NODES] = x
    maps1 = _layer_inputs(plan, xp, np.asarray(W1), np.asarray(a_src1),
                          np.asarray(a_dst1))
    o1, res1 = _run_layer(nc, maps1, trace=_trace)

    h1 = np.maximum(o1[:, IPERM] + np.asarray(b1, np.float32), 0.0)
    h1[N_NODES:] = 0.0
    maps2 = _layer_inputs(plan, h1, np.asarray(W2), np.asarray(a_src2),
                          np.asarray(a_dst2))
    o2, res2 = _run_layer(nc, maps2, trace=_trace)

    if _collect is not None:
        _collect.extend([res1, res2])
    return np.maximum(o2[:N_NODES][:, IPERM] + np.asarray(b2, np.float32),
                      0.0).astype(np.float32)
